# revision 6
# baseline (speedup 1.0000x reference)
"""Multi-head attention Trainium2 Bass kernel.

Problem: x:(4,512,1024), Wq/Wk/Wv/Wo:(512,512), H=8 heads, d=64.
  q = Wq@x ; k = Wk@x ; v = Wv@x  (per batch, 1x1 conv == channel matmul)
  per head: S[i,j] = q[:,i].k[:,j] ; attn = softmax_j(S) ; y = attn @ v
  out = Wo @ y

Sharding: 8 cores = (batch b, head-half g).  Core (b,g) handles batch b,
local heads g*4..g*4+3.  Each core computes a partial output
out_p = Wo[:, g*256:(g+1)*256] @ y_g  which the host sums pairwise.

Per-core layout trick: scores are computed TRANSPOSED (S^T[j,i] = k^T q)
so no PE transposes are needed anywhere:
  - q,k projections: lhsT = W^T slices (host pre-transposed), rhs = x.
  - v^T computed directly: lhsT = x tiles, rhs = Wv^T.
  - S^T: lhsT = k (d=64 on partitions), rhs = q.  Heads are processed in
    pairs living on partition halves 0:64 / 64:128 -> the two K=64 matmuls
    use distinct PE row-groups and run concurrently.
  - softmax without max subtraction (scores max ~52 < 88 overflow limit),
    exp on ACT straight out of PSUM.
  - PV: lhsT = [v^T | ones] (128,65) -> row 64 of the product is the
    softmax denominator for free.
  - normalize: r = 1/rowsum broadcast across partitions via a DRAM
    round-trip DMA, y = y_unnorm * r on DVE.
  - out projection: lhsT = Wo^T slices per head (K=64).

All matmuls run in float32r (full PE rate for free-dim >= 256).
"""

import numpy as np

import concourse.bass as bass
import concourse.tile as tile
from concourse import bacc
from concourse import mybir
from concourse.bass_utils import run_bass_kernel_spmd

F32 = mybir.dt.float32
F32R = mybir.dt.float32r

P = 128
C = 512          # channels
NSEQ = 1024      # sequence length
D = 64           # head dim
HL = 4           # local heads per core
KC = C // P      # 4 contraction tiles over channels
J = NSEQ // P    # 8 key tiles
NH = NSEQ // 512  # 2 query-half chunks (PSUM bank = 512 fp32)

_NC_CACHE = {}


def build_nc():
    nc = bacc.Bacc("TRN2")

    x = nc.dram_tensor("x", [C, NSEQ], F32R, kind="ExternalInput")
    wq = nc.dram_tensor("wq_t", [C, 2 * P], F32R, kind="ExternalInput")
    wk = nc.dram_tensor("wk_t", [C, 2 * P], F32R, kind="ExternalInput")
    wv = nc.dram_tensor("wv_t", [C, 2 * P], F32R, kind="ExternalInput")
    wo = nc.dram_tensor("wo_t", [D, HL, C], F32R, kind="ExternalInput")
    out = nc.dram_tensor("out_p", [C, NSEQ], F32, kind="ExternalOutput")

    with tile.TileContext(nc) as tc:
        with (
            tc.tile_pool(name="consts", bufs=1) as consts,
            tc.tile_pool(name="epool", bufs=3) as epool,
            tc.tile_pool(name="rpool", bufs=2) as rpool,
            tc.tile_pool(name="opool", bufs=3) as opool,
            tc.tile_pool(name="drams", bufs=1, space="DRAM") as drams,
            tc.tile_pool(name="pmm", bufs=2, space="PSUM") as pmm,
            tc.tile_pool(name="ps", bufs=2, space="PSUM") as psp,
            tc.tile_pool(name="py", bufs=1, space="PSUM") as pyp,
        ):
            # ---- load inputs ----
            x_sb = consts.tile([P, KC, NSEQ], F32R)
            nc.sync.dma_start(x_sb, x.rearrange("(kc p) n -> p kc n", p=P))
            wq_sb = consts.tile([P, KC, 2 * P], F32R)
            nc.sync.dma_start(wq_sb, wq.rearrange("(kc p) m -> p kc m", p=P))
            wk_sb = consts.tile([P, KC, 2 * P], F32R)
            nc.sync.dma_start(wk_sb, wk.rearrange("(kc p) m -> p kc m", p=P))
            wv_sb = consts.tile([P, KC, 2 * P], F32R)
            nc.sync.dma_start(wv_sb, wv.rearrange("(kc p) m -> p kc m", p=P))
            wot_sb = consts.tile([D, HL, C], F32R)
            nc.sync.dma_start(wot_sb, wo[:, :, :])

            # ---- q, k projections: (256,512)@(512,1024) each ----
            q_sb = consts.tile([P, 2, NSEQ], F32R)
            k_sb = consts.tile([P, 2, NSEQ], F32R)
            for w_sb, dst in ((wq_sb, q_sb), (wk_sb, k_sb)):
                for m in range(2):
                    for nn in range(NH):
                        ps = pmm.tile([P, 512], F32, tag="mm")
                        for kc in range(KC):
                            nc.tensor.matmul(
                                ps,
                                lhsT=(w_sb[:, kc, m * P:(m + 1) * P]),
                                rhs=(x_sb[:, kc, nn * 512:(nn + 1) * 512]),
                                start=(kc == 0),
                                stop=(kc == KC - 1),
                            )
                        nc.scalar.copy(
                            out=dst[:, m, nn * 512:(nn + 1) * 512], in_=ps
                        )

            # ---- v^T projection: out[j, d'] tiles, plus ones column ----
            vt_sb = consts.tile([P, J, HL, D + 1], F32R)
            # memset can't target f32r; zero the ones-column via a uint32
            # view, then produce rounded-f32r 1.0s with ACT (0*x + 1).
            ones_col = vt_sb[:, :, :, D:D + 1]
            nc.vector.memset(ones_col.bitcast(mybir.dt.uint32), 0)
            nc.scalar.activation(
                out=ones_col, in_=ones_col,
                func=mybir.ActivationFunctionType.Identity,
                bias=1.0, scale=0.0,
            )
            for j in range(J):
                psv = pmm.tile([P, 512], F32, tag="mm")
                for kc in range(KC):
                    nc.tensor.matmul(
                        psv[:, 0:2 * P],
                        lhsT=(x_sb[:, kc, j * P:(j + 1) * P]),
                        rhs=(wv_sb[:, kc, :]),
                        start=(kc == 0),
                        stop=(kc == KC - 1),
                    )
                for h in range(HL):
                    nc.vector.tensor_copy(
                        out=vt_sb[:, j, h, 0:D], in_=psv[:, h * D:(h + 1) * D]
                    )

            # ---- attention, head pairs on partition halves ----
            y_sb = consts.tile([D, HL, NSEQ], F32R)
            r_dram = drams.tile([HL, NH, 512], F32)
            for pair in range(2):
                for nn in range(NH):
                    isl = slice(nn * 512, (nn + 1) * 512)
                    py0 = pyp.tile([P, 512], F32, tag="py0")
                    py1 = pyp.tile([P, 512], F32, tag="py1")
                    for j in range(J):
                        ps0 = psp.tile([P, 512], F32, tag="s0")
                        ps1 = psp.tile([P, 512], F32, tag="s1")
                        # S^T tiles for the two heads (distinct PE row groups)
                        nc.tensor.matmul(
                            ps0,
                            lhsT=(k_sb[0:D, pair, j * P:(j + 1) * P]),
                            rhs=(q_sb[0:D, pair, isl]),
                            start=True, stop=True,
                        )
                        nc.tensor.matmul(
                            ps1,
                            lhsT=(k_sb[D:P, pair, j * P:(j + 1) * P]),
                            rhs=(q_sb[D:P, pair, isl]),
                            start=True, stop=True,
                        )
                        e0 = epool.tile([P, 512], F32R, tag="e0")
                        e1 = epool.tile([P, 512], F32R, tag="e1")
                        nc.scalar.activation(
                            out=e0, in_=ps0, func=mybir.ActivationFunctionType.Exp
                        )
                        nc.scalar.activation(
                            out=e1, in_=ps1, func=mybir.ActivationFunctionType.Exp
                        )
                        # PV accumulate; lhsT has the ones column -> row D is
                        # the softmax denominator.
                        nc.tensor.matmul(
                            py0[0:D + 1, :],
                            lhsT=(vt_sb[:, j, 2 * pair, :]),
                            rhs=(e0),
                            start=(j == 0), stop=(j == J - 1),
                        )
                        nc.tensor.matmul(
                            py1[0:D + 1, :],
                            lhsT=(vt_sb[:, j, 2 * pair + 1, :]),
                            rhs=(e1),
                            start=(j == 0), stop=(j == J - 1),
                        )
                    for idx, py in ((0, py0), (1, py1)):
                        h = 2 * pair + idx
                        rsb = rpool.tile([P, 512], F32, tag="rsb")
                        nc.vector.reciprocal(
                            out=rsb[D:D + 1, :], in_=py[D:D + 1, :]
                        )
                        nc.sync.dma_start(
                            out=r_dram[h, nn:nn + 1, :], in_=rsb[D:D + 1, :]
                        )
                        rb = r_dram[h, nn:nn + 1, :]
                        bcast = bass.AP(
                            tensor=rb.tensor, offset=rb.offset,
                            ap=[[0, D], [1, 512]],
                        )
                        rr = rpool.tile([D, 512], F32, tag="rr")
                        nc.sync.dma_start(out=rr, in_=bcast)
                        nc.vector.tensor_tensor(
                            out=y_sb[:, h, isl], in0=py[0:D, :], in1=rr,
                            op=mybir.AluOpType.mult,
                        )

            # ---- output projection: (512,256)@(256,1024) as 4 K=64 tiles ----
            out_t = out.rearrange("(m p) n -> p m n", p=P)
            for m in range(4):
                for nn in range(NH):
                    po = pmm.tile([P, 512], F32, tag="mm")
                    for h in range(HL):
                        nc.tensor.matmul(
                            po,
                            lhsT=(wot_sb[:, h, m * P:(m + 1) * P]),
                            rhs=(y_sb[:, h, nn * 512:(nn + 1) * 512]),
                            start=(h == 0),
                            stop=(h == HL - 1),
                        )
                    ot = opool.tile([P, 512], F32, tag="ot")
                    nc.scalar.copy(out=ot, in_=po)
                    nc.sync.dma_start(
                        out=out_t[:, m, nn * 512:(nn + 1) * 512], in_=ot
                    )

    nc.compile()
    return nc


def get_nc():
    if "nc" not in _NC_CACHE:
        _NC_CACHE["nc"] = build_nc()
    return _NC_CACHE["nc"]


def make_in_maps(x, Wq, Wk, Wv, Wo):
    in_maps = []
    for core in range(8):
        b, g = core // 2, core % 2
        sl = slice(g * 256, (g + 1) * 256)
        in_maps.append({
            "x": np.ascontiguousarray(x[b]),
            "wq_t": np.ascontiguousarray(Wq[sl, :].T),
            "wk_t": np.ascontiguousarray(Wk[sl, :].T),
            "wv_t": np.ascontiguousarray(Wv[sl, :].T),
            # [d, h, o] so lhsT slices are contiguous per head
            "wo_t": np.ascontiguousarray(
                Wo[:, sl].reshape(C, HL, D).transpose(2, 1, 0)
            ),
        })
    return in_maps


LAST_RESULTS = {}


def kernel(x, Wq, Wk, Wv, Wo, _trace=False):
    x = np.asarray(x, dtype=np.float32)
    Wq = np.asarray(Wq, dtype=np.float32)
    Wk = np.asarray(Wk, dtype=np.float32)
    Wv = np.asarray(Wv, dtype=np.float32)
    Wo = np.asarray(Wo, dtype=np.float32)

    nc = get_nc()
    in_maps = make_in_maps(x, Wq, Wk, Wv, Wo)
    res = run_bass_kernel_spmd(
        nc, in_maps, core_ids=list(range(8)), trace=_trace
    )
    LAST_RESULTS["res"] = res
    parts = [np.asarray(r["out_p"]) for r in res.results]
    out = np.stack([parts[2 * b] + parts[2 * b + 1] for b in range(4)])
    return out
